# revision 26
# baseline (speedup 1.0000x reference)
"""Dense transformer block (B=4,S=2048,E=1024,H=16) on 8 trn2 cores.

Sharding: 2 cores per batch sequence; core parity p takes rows p, p+2, ...
(stride-2 interleave) as its query rows -- this balances causal-attention
work exactly across cores.  Each core's x input is row-permuted to
[q rows (local order), other-parity rows] so every SBUF offset in the
SPMD program is compile-time constant; causality is enforced with per-core
0/1 mask tensors (pure data).

All matmul operands are bf16 (full PE rate, half the DMA + SBUF of f32),
accumulation stays fp32 in PSUM.  K^T/Q^T/V/O all live in SBUF -- no DRAM
round trips for attention; FFN weights stream from HBM, overlapped.
"""

import numpy as np

B, S, E, H, DH = 4, 2048, 1024, 16, 64
EPS = 1e-5
QR = S // 2          # q rows per core
CH = 512             # q-chunk (matmul free dim)
NCH = QR // CH       # 2 chunks
NKB = S // 128       # 16 key blocks
ET = E // 128        # 8 E tiles
NPR = H // 2         # 8 head pairs
FE = 4 * E           # ffn hidden
NS4 = FE // 128      # 32 ffn hidden slices
SC = 1.0 / np.sqrt(DH)

# visit tables: per q-chunk, which key-blocks are fully visible / diagonal
FULL_KB = {0: [], 1: [0, 1, 2, 3, 8, 9, 10, 11]}
DIAG_KB = {0: [0, 1, 2, 3, 8, 9, 10, 11], 1: [4, 5, 6, 7, 12, 13, 14, 15]}

_PROG = None


def _build():
    import concourse.bacc as bacc
    import concourse.tile as tile
    from concourse import mybir
    from concourse.masks import make_identity

    F32 = mybir.dt.float32
    F32R = mybir.dt.float32r
    BF16 = mybir.dt.bfloat16
    AF = mybir.ActivationFunctionType
    SUB = mybir.AluOpType.subtract
    MULT = mybir.AluOpType.mult
    ADD = mybir.AluOpType.add

    nc = bacc.Bacc("TRN2", target_bir_lowering=False, debug=False, num_devices=8)

    xbf = nc.dram_tensor("xbf", [S, E], BF16, kind="ExternalInput").ap()
    xq = nc.dram_tensor("xq", [QR, E], F32, kind="ExternalInput").ap()
    masks2 = nc.dram_tensor("masks2", [2, 128, 2, 128], BF16, kind="ExternalInput").ap()
    wqh = nc.dram_tensor("wqh", [NPR, 128, ET, 128], BF16, kind="ExternalInput").ap()
    wkh = nc.dram_tensor("wkh", [NPR, 128, ET, 128], BF16, kind="ExternalInput").ap()
    wvh = nc.dram_tensor("wvh", [2, 128, ET, 8, DH], BF16, kind="ExternalInput").ap()
    woh = nc.dram_tensor("woh", [128, NPR, E], BF16, kind="ExternalInput").ap()
    w1h = nc.dram_tensor("w1h", [NS4, 128, ET, 128], BF16, kind="ExternalInput").ap()
    w2h = nc.dram_tensor("w2h", [2, NS4, 128, 512], BF16, kind="ExternalInput").ap()
    b2r = nc.dram_tensor("b2r", [1, E], BF16, kind="ExternalInput").ap()
    ln1g = nc.dram_tensor("ln1g", [128, ET], F32, kind="ExternalInput").ap()
    ln1b = nc.dram_tensor("ln1b", [128, ET], F32, kind="ExternalInput").ap()
    ln2g = nc.dram_tensor("ln2g", [128, ET], F32, kind="ExternalInput").ap()
    ln2b = nc.dram_tensor("ln2b", [128, ET], F32, kind="ExternalInput").ap()
    b1h = nc.dram_tensor("b1h", [128, NS4], F32, kind="ExternalInput").ap()
    out = nc.dram_tensor("out", [QR, E], F32, kind="ExternalOutput").ap()

    with tile.TileContext(nc, pool_alloc_mode="queue") as tc:
        consts = tc.alloc_tile_pool(name="consts", bufs=1)

        ident = consts.tile([128, 128], BF16, tag="ident")
        make_identity(nc, ident)
        onesb = consts.tile([1, 128], BF16, tag="onesb")
        nc.vector.memset(onesb, 1.0)
        epst = consts.tile([128, 1], F32, tag="eps")
        nc.vector.memset(epst, EPS)
        ln1g_sb = consts.tile([128, ET], F32, tag="lnp1")
        nc.sync.dma_start(ln1g_sb, ln1g)
        ln1b_sb = consts.tile([128, ET], F32, tag="lnp2")
        nc.sync.dma_start(ln1b_sb, ln1b)
        ln2g_sb = consts.tile([128, ET], F32, tag="lnp3")
        nc.sync.dma_start(ln2g_sb, ln2g)
        ln2b_sb = consts.tile([128, ET], F32, tag="lnp4")
        nc.sync.dma_start(ln2b_sb, ln2b)
        b1_sb = consts.tile([128, NS4], F32, tag="b1")
        nc.sync.dma_start(b1_sb, b1h)
        b2_sb = consts.tile([1, E], BF16, tag="b2")
        nc.sync.dma_start(b2_sb, b2r)
        wedges = []
        for w in range(2):
            mt = consts.tile([128, 2, 128], BF16, tag=f"wed{w}")
            nc.sync.dma_start(mt, masks2[w])
            wedges.append(mt)

        # persistent SBUF arrays (alloc order = reverse release order)
        ktp = tc.alloc_tile_pool(name="ktp", bufs=NPR)
        KT = [ktp.tile([128, S], BF16, tag="kt", name=f"KT{i}") for i in range(NPR)]
        qtp = tc.alloc_tile_pool(name="qtp", bufs=NPR)
        QT = [qtp.tile([128, QR], BF16, tag="qt", name=f"QT{i}") for i in range(NPR)]
        vap = tc.alloc_tile_pool(name="vap", bufs=NKB)
        VA = [vap.tile([128, H, DH + 1], BF16, tag="va", name=f"VA{i}")
              for i in range(NKB)]
        y2p = tc.alloc_tile_pool(name="y2p", bufs=ET, side="right")
        Y2T = [y2p.tile([128, QR], BF16, tag="y2t", name=f"Y2T{i}")
               for i in range(ET)]
        oap = tc.alloc_tile_pool(name="oap", bufs=NPR)
        OACC = [oap.tile([128, QR], BF16, tag="oacc", name=f"OACC{i}")
                for i in range(NPR)]
        dramp = tc.alloc_tile_pool(name="dramp", bufs=1, space="DRAM")
        X2D = dramp.tile([QR, E], F32, name="X2D")

        for kb in range(NKB):
            nc.vector.memset(VA[kb][:, :, DH], 1.0)  # softmax-denominator column

        # ---------------- P1: LN1 + transpose + QKV projections, streamed ----
        with (
            tc.tile_pool(name="wqkv", bufs=1) as wqkv,
            tc.tile_pool(name="xp", bufs=8) as xp,
            tc.tile_pool(name="y1p", bufs=2) as y1p,
            tc.tile_pool(name="psT", bufs=2, space="PSUM") as psT,
            tc.tile_pool(name="psP", bufs=3, space="PSUM") as psP,
        ):
            wvs = []
            for half in range(2):
                wt = wqkv.tile([128, ET, 8, DH], BF16, tag="wv", name=f"wv{half}",
                               bufs=2)
                nc.sync.dma_start(wt, wvh[half])
                wvs.append(wt)

            for tq in range(4):
                xts = []
                mvall = xp.tile([128, 4, 2], F32, tag="bnmv", bufs=2)
                for j in range(4):
                    xt = xp.tile([128, E], BF16, tag="xt", name=f"xt{j}", bufs=6)
                    nc.sync.dma_start(
                        xt, xbf[(tq * 4 + j) * 128:(tq * 4 + j + 1) * 128, :])
                    st = xp.tile([128, 2, 6], F32, tag="bnst", bufs=4)
                    xr = xt.rearrange("p (a b) -> p a b", a=2)
                    for sg in range(2):
                        nc.vector.bn_stats(st[:, sg, :], xr[:, sg, :])
                    nc.vector.bn_aggr(mvall[:, j, :], st)
                    xts.append(xt)
                sq = xp.tile([128, 4], F32, tag="sq", bufs=2)
                nc.scalar.activation(sq, mvall[:, :, 1], AF.Sqrt, bias=epst)
                rstd = xp.tile([128, 4], F32, tag="rstd", bufs=2)
                nc.vector.reciprocal_approx_fast(rstd, sq)
                for j in range(4):
                    nc.vector.tensor_scalar(
                        xts[j], xts[j], mvall[:, j, 0:1], rstd[:, j:j + 1],
                        SUB, MULT)
                if tq == 0:
                    # warm the HAM clock gate between the LN head and the
                    # first projection matmuls (depends on xts[0] so it
                    # doesn't fire too early and re-throttle before use)
                    warm = psT.tile([128, 1024], BF16, tag="pst")
                    for _ in range(80):
                        nc.tensor.transpose(warm[:, 0:128],
                                            xts[0][:, 0:128], ident)
                y1t = y1p.tile([128, ET, 512], BF16, tag="y1t")
                for e in range(ET):
                    pst = psT.tile([128, 1024], BF16, tag="pst")
                    for j in range(4):
                        nc.tensor.transpose(
                            pst[:, j * 128:(j + 1) * 128],
                            xts[j][:, e * 128:(e + 1) * 128], ident)
                    nc.vector.tensor_scalar(
                        y1t[:, e, :], pst[:, 0:512],
                        ln1g_sb[:, e:e + 1], ln1b_sb[:, e:e + 1], MULT, ADD)
                # K projection for this 512-col block of all head pairs
                for pr in range(NPR):
                    wk_t = wqkv.tile([128, ET, 128], BF16, tag="wk", bufs=4)
                    nc.sync.dma_start(wk_t, wkh[pr])
                    ps = psP.tile([128, 512], F32, tag="pp")
                    for e in range(ET):
                        nc.tensor.matmul(ps, wk_t[:, e, :], y1t[:, e, :],
                                         start=(e == 0), stop=(e == ET - 1))
                    nc.scalar.copy(KT[pr][:, tq * 512:(tq + 1) * 512], ps)
                # Q projection (first two 512-blocks are the q rows)
                if tq < NCH:
                    for pr in range(NPR):
                        wq_t = wqkv.tile([128, ET, 128], BF16, tag="wq", bufs=4)
                        nc.sync.dma_start(wq_t, wqh[pr])
                        ps = psP.tile([128, 512], F32, tag="pp")
                        for e in range(ET):
                            nc.tensor.matmul(ps, wq_t[:, e, :], y1t[:, e, :],
                                             start=(e == 0), stop=(e == ET - 1))
                        nc.vector.tensor_copy(QT[pr][:, tq * 512:(tq + 1) * 512], ps)
                # V projection (natural layout) for the 4 key blocks here
                for kbl in range(4):
                    kb = 4 * tq + kbl
                    for half in range(2):
                        ps = psP.tile([128, 512], F32, tag="pp")
                        for e in range(ET):
                            nc.tensor.matmul(
                                ps, y1t[:, e, kbl * 128:(kbl + 1) * 128],
                                wvs[half][:, e, :, :],
                                start=(e == 0), stop=(e == ET - 1))
                        nc.scalar.copy(
                            VA[kb][:, 8 * half:8 * half + 8, 0:DH],
                            ps.rearrange("p (h d) -> p h d", h=8))

        # ---------------- P2: attention;  P3: out proj;  P4: LN2 ----------
        with tc.tile_pool(name="wop", bufs=1) as wop:
            wo_sb = wop.tile([128, NPR, E], BF16, tag="wo")
            nc.sync.dma_start(wo_sb, woh)

            with (
                tc.tile_pool(name="ptp", bufs=4) as ptp,
                tc.tile_pool(name="nrm", bufs=2) as nrm,
                tc.tile_pool(name="psS", bufs=2, space="PSUM") as psS,
                tc.tile_pool(name="psO", bufs=4, space="PSUM") as psO,
            ):
                def do_norm(ops, ch, pr):
                    # normalize: o / rowsum -> OACC.  Runs inline at each
                    # stream's pair boundary; the other stream's visits keep
                    # the PE busy during this vector chain.
                    rs = nrm.tile([1, 1024], F32, tag="rs")
                    for hh in range(2):
                        nc.vector.tensor_copy(
                            rs[0:1, hh * 512:(hh + 1) * 512],
                            ops[hh][DH:DH + 1, :])
                    rcf = nrm.tile([1, 1024], F32, tag="rcf")
                    nc.vector.reciprocal_approx_fast(rcf, rs)
                    bcs = nrm.tile([64, 1024], F32, tag="bcs")
                    nc.gpsimd.partition_broadcast(bcs, rcf[0:1, :])
                    for hh in range(2):
                        nc.vector.tensor_mul(
                            OACC[pr][hh * 64:(hh + 1) * 64,
                                     ch * 512:(ch + 1) * 512],
                            ops[hh][0:DH, :], bcs[:, hh * 512:(hh + 1) * 512])

                def do_post(item):
                    ops, ch, pr, kb, q0, N, pss, pt, first, last = item
                    if N == 512:
                        nc.scalar.activation(pt, pss, AF.Exp, scale=SC)
                    else:
                        pss3 = pss.rearrange("p (a b) -> p a b", a=2)[:, :, 0:N]
                        pt3 = pt.rearrange("p (a b) -> p a b", a=2)[:, :, 0:N]
                        nc.scalar.activation(pt3, pss3, AF.Exp, scale=SC)
                    if kb in DIAG_KB[ch]:
                        ptm = pt.rearrange("p (a b) -> p a b", a=2)[:, :, 0:128]
                        nc.vector.tensor_mul(ptm, ptm,
                                             wedges[0 if kb < 8 else 1])
                    for hh in range(2):
                        nc.tensor.matmul(
                            ops[hh][0:DH + 1, q0:512],
                            VA[kb][:, 2 * pr + hh, :],
                            pt[:, hh * 512:hh * 512 + N],
                            start=first, stop=last, skip_group_check=True)

                # interleave the ch0 stream (8 sparse diag visits per pair)
                # with the ch1 stream (16 visits per pair) 1:2 so the PE
                # never starves; each stream owns one pair of psO banks.
                visits1 = FULL_KB[1] + DIAG_KB[1]
                A = [(0, k // 8, k % 8, DIAG_KB[0][k % 8]) for k in range(64)]
                Bv = [(1, k // 16, k % 16, visits1[k % 16]) for k in range(128)]
                seq, DLY = [], 4  # stagger A half a pair so norms don't collide
                for k in range(64 + DLY):
                    if k < 64:
                        seq.append(Bv[2 * k])
                        seq.append(Bv[2 * k + 1])
                    if k >= DLY:
                        seq.append(A[k - DLY])
                nv_of = {0: 8, 1: 16}
                ops_live = {0: None, 1: None}
                pending = None
                for ch, pr, vi, kb in seq:
                    diag = kb in DIAG_KB[ch]
                    q0 = 128 * (kb % 4) if diag else 0
                    N = 512 - q0
                    if vi == 0:
                        if ops_live[ch] is not None:
                            do_norm(ops_live[ch], ch, pr - 1)
                        ops_live[ch] = [
                            psO.tile([128, 512], F32, tag="ot", name=f"ot{h}")
                            for h in range(2)]
                    ops = ops_live[ch]
                    pss = psS.tile([128, 1024], F32, tag="sc")
                    for hh in range(2):
                        nc.tensor.matmul(
                            pss[:, hh * 512:hh * 512 + N],
                            KT[pr][hh * 64:(hh + 1) * 64,
                                   kb * 128:(kb + 1) * 128],
                            QT[pr][hh * 64:(hh + 1) * 64,
                                   ch * 512 + q0:(ch + 1) * 512],
                            start=True, stop=True)
                    pt = ptp.tile([128, 1024], BF16, tag="pt", bufs=4)
                    if pending is not None:
                        do_post(pending)
                    pending = (ops, ch, pr, kb, q0, N, pss, pt,
                               vi == 0, vi == nv_of[ch] - 1)
                do_post(pending)
                do_norm(ops_live[0], 0, NPR - 1)
                do_norm(ops_live[1], 1, NPR - 1)

            # ---- P3 (out projection + residual) with inline P4 (LN2) ----
            with (
                tc.tile_pool(name="xqp", bufs=2) as xqp,
                tc.tile_pool(name="x2tp", bufs=6) as x2tp,
                tc.tile_pool(name="n2p", bufs=4) as n2p,
                tc.tile_pool(name="st2", bufs=4) as st2,
                tc.tile_pool(name="psP3", bufs=3, space="PSUM") as psP3,
                tc.tile_pool(name="psT2", bufs=2, space="PSUM") as psT2,
            ):
                x2group = []

                def ln2_half(tq):
                    n2s = []
                    mvall = st2.tile([128, 4, 2], F32, tag="bnmv2", bufs=2)
                    for j in range(4):
                        x2t = x2group[j]
                        st = st2.tile([128, 2, 6], F32, tag="bnst2")
                        x2r = x2t.rearrange("p (a b) -> p a b", a=2)
                        for sg in range(2):
                            nc.vector.bn_stats(st[:, sg, :], x2r[:, sg, :])
                        nc.vector.bn_aggr(mvall[:, j, :], st)
                    sq = st2.tile([128, 4], F32, tag="sq2", bufs=2)
                    nc.scalar.activation(sq, mvall[:, :, 1], AF.Sqrt, bias=epst)
                    rstd = st2.tile([128, 4], F32, tag="rstd2", bufs=2)
                    nc.vector.reciprocal_approx_fast(rstd, sq)
                    for j in range(4):
                        n2 = n2p.tile([128, E], BF16, tag="n2", name=f"n2{j}")
                        nc.vector.tensor_scalar(
                            n2, x2group[j], mvall[:, j, 0:1], rstd[:, j:j + 1],
                            SUB, MULT)
                        n2s.append(n2)
                    for e in range(ET):
                        pst = psT2.tile([128, 1024], BF16, tag="pst2")
                        for j in range(4):
                            nc.tensor.transpose(
                                pst[:, j * 128:(j + 1) * 128],
                                n2s[j][:, e * 128:(e + 1) * 128], ident)
                        nc.vector.tensor_scalar(
                            Y2T[e][:, tq * 512:(tq + 1) * 512], pst[:, 0:512],
                            ln2g_sb[:, e:e + 1], ln2b_sb[:, e:e + 1], MULT, ADD)

                for qt in range(ET):
                    xq_t = xqp.tile([128, E], F32, tag="xq")
                    nc.sync.dma_start(xq_t, xq[qt * 128:(qt + 1) * 128, :])
                    x2t = x2tp.tile([128, E], F32, tag="x2t")
                    for eh in range(2):
                        ps = psP3.tile([128, 512], F32, tag="po")
                        for pr in range(NPR):
                            nc.tensor.matmul(
                                ps, OACC[pr][:, qt * 128:(qt + 1) * 128],
                                wo_sb[:, pr, eh * 512:(eh + 1) * 512],
                                start=(pr == 0), stop=(pr == NPR - 1))
                        nc.vector.tensor_add(
                            x2t[:, eh * 512:(eh + 1) * 512], ps,
                            xq_t[:, eh * 512:(eh + 1) * 512])
                    nc.sync.dma_start(X2D[qt * 128:(qt + 1) * 128, :], x2t)
                    x2group.append(x2t)
                    if qt == 3:
                        ln2_half(0)
                        x2group = []
                ln2_half(1)

        oap.release()
        vap.release()
        qtp.release()
        ktp.release()

        # ---------------- P5: FFN + residual -> out ----------------
        with (
            tc.tile_pool(name="h1p", bufs=NS4) as h1p,
            tc.tile_pool(name="w1p", bufs=8) as w1p,
            tc.tile_pool(name="w2p", bufs=32) as w2p,
            tc.tile_pool(name="otp", bufs=4) as otp,
            tc.tile_pool(name="x2rp", bufs=8) as x2rp,
            tc.tile_pool(name="psF1", bufs=2, space="PSUM") as psF1,
            tc.tile_pool(name="psF2", bufs=4, space="PSUM") as psF2,
        ):
            for qh in range(NCH):
                h1t = []
                for s4 in range(NS4):
                    w1s = w1p.tile([128, ET, 128], BF16, tag="w1")
                    nc.sync.dma_start(w1s, w1h[s4])
                    ps = psF1.tile([128, 512], F32, tag="f1")
                    for e in range(ET):
                        nc.tensor.matmul(
                            ps, w1s[:, e, :], Y2T[e][:, qh * 512:(qh + 1) * 512],
                            start=(e == 0), stop=(e == ET - 1))
                    ht = h1p.tile([128, 512], BF16, tag="h1")
                    nc.scalar.activation(ht, ps, AF.Relu,
                                         bias=b1_sb[:, s4:s4 + 1])
                    h1t.append(ht)
                for eh in range(2):
                    x2rs = []
                    for ql in range(4):
                        qt = qh * 4 + ql
                        x2r = x2rp.tile([128, 512], F32, tag="x2r")
                        nc.sync.dma_start(
                            x2r, X2D[qt * 128:(qt + 1) * 128,
                                     eh * 512:(eh + 1) * 512])
                        x2rs.append(x2r)
                    pss_t = [psF2.tile([128, 512], F32, tag="f2", name=f"f2_{q}")
                             for q in range(4)]
                    for s4 in range(NS4):
                        w2s = w2p.tile([128, 512], BF16, tag="w2")
                        nc.sync.dma_start(w2s, w2h[eh, s4])
                        for ql in range(4):
                            nc.tensor.matmul(
                                pss_t[ql], h1t[s4][:, ql * 128:(ql + 1) * 128],
                                w2s, start=(s4 == 0), stop=False)
                    for ql in range(4):
                        qt = qh * 4 + ql
                        nc.tensor.matmul(
                            pss_t[ql], onesb, b2_sb[0:1, eh * 512:(eh + 1) * 512],
                            start=False, stop=True)
                        ot = otp.tile([128, 512], F32, tag="ot")
                        nc.vector.tensor_add(ot, pss_t[ql], x2rs[ql])
                        nc.sync.dma_start(
                            out[qt * 128:(qt + 1) * 128,
                                eh * 512:(eh + 1) * 512], ot)

        y2p.release()
        dramp.release()
        consts.release()

    nc.compile()
    return nc


def _prep_inputs(inputs):
    import ml_dtypes
    bf16 = ml_dtypes.bfloat16

    x = np.ascontiguousarray(inputs["x"], dtype=np.float32)
    wq = np.ascontiguousarray(inputs["Wq"], np.float32)   # [H, E, DH]
    wk = np.ascontiguousarray(inputs["Wk"], np.float32)
    wv = np.ascontiguousarray(inputs["Wv"], np.float32)
    wo = np.ascontiguousarray(inputs["Wo"], np.float32)   # [E, E]
    bo = np.ascontiguousarray(inputs["bo"], np.float32)
    w1 = np.ascontiguousarray(inputs["W1"], np.float32)   # [E, 4E]
    b1 = np.ascontiguousarray(inputs["b1"], np.float32)
    w2 = np.ascontiguousarray(inputs["W2"], np.float32)   # [4E, E]
    b2 = np.ascontiguousarray(inputs["b2"], np.float32)

    def qkv_pack(w, group):  # [H, E, DH] -> [ngrp, 128, ET, group*DH]
        n = H // group
        a = w.reshape(n, group, ET, 128, DH)          # [n, g, eo, ei, d]
        a = a.transpose(0, 3, 2, 1, 4)                # [n, ei, eo, g, d]
        return np.ascontiguousarray(
            a.reshape(n, 128, ET, group * DH)).astype(bf16)

    shared = {
        "wqh": qkv_pack(wq, 2),
        "wkh": qkv_pack(wk, 2),
        "wvh": qkv_pack(wv, 8),
        "woh": np.ascontiguousarray(
            wo.reshape(NPR, 128, E).transpose(1, 0, 2)).astype(bf16),
        "w1h": np.ascontiguousarray(
            w1.reshape(ET, 128, NS4, 128).transpose(2, 1, 0, 3)).astype(bf16),
        "w2h": np.ascontiguousarray(
            w2.reshape(NS4, 128, 2, 512).transpose(2, 0, 1, 3)).astype(bf16),
        "b2r": b2.reshape(1, E).astype(bf16),
        "b1h": np.ascontiguousarray(b1.reshape(NS4, 128).T),
        "ln1g": np.ascontiguousarray(inputs["ln1_g"].reshape(ET, 128).T),
        "ln1b": np.ascontiguousarray(inputs["ln1_b"].reshape(ET, 128).T),
        "ln2g": np.ascontiguousarray(inputs["ln2_g"].reshape(ET, 128).T),
        "ln2b": np.ascontiguousarray(inputs["ln2_b"].reshape(ET, 128).T),
    }
    kk = np.arange(128)[:, None]
    qq = np.arange(128)[None, :]
    in_maps = []
    for c in range(8):
        b, p = c // 2, c % 2
        perm = np.concatenate([np.arange(p, S, 2), np.arange(1 - p, S, 2)])
        m = np.zeros((2, 128, 128), np.float32)
        m[0] = (qq >= kk).astype(np.float32)          # own-parity blocks
        if p == 0:
            m[1] = (qq > kk).astype(np.float32)       # other-parity, even core
        else:
            m[1] = (qq >= kk).astype(np.float32)      # other-parity, odd core
        im = dict(shared)
        im["xbf"] = np.ascontiguousarray(x[b][perm]).astype(bf16)
        im["xq"] = np.ascontiguousarray(x[b][perm[:QR]] + bo[None, :])
        im["masks2"] = np.ascontiguousarray(
            np.broadcast_to(m[:, :, None, :], (2, 128, 2, 128))).astype(bf16)
        in_maps.append(im)
    return in_maps


def _get_prog():
    global _PROG
    if _PROG is None:
        _PROG = _build()
    return _PROG


def run(inputs, trace=False):
    from concourse.bass_utils import run_bass_kernel_spmd

    nc = _get_prog()
    in_maps = _prep_inputs(inputs)
    kw = {}
    if trace:
        import sys, types
        try:
            from antenv.axon_hooks import get_axon_ntff_profile_hook  # noqa
        except ImportError:
            from trn_agent_boot.trn_boot import _ntff_profile_via_ctypes
            hook = _ntff_profile_via_ctypes("/opt/axon/libaxon_pjrt.so")
            mod = types.ModuleType("antenv.axon_hooks")
            mod.get_axon_ntff_profile_hook = lambda: hook
            sys.modules["antenv.axon_hooks"] = mod
        kw["trace"] = True
    res = run_bass_kernel_spmd(nc, in_maps, core_ids=list(range(8)), **kw)
    outp = np.empty((B, S, E), np.float32)
    for c in range(8):
        b, p = c // 2, c % 2
        outp[b, p::2, :] = res.results[c]["out"]
    return outp, res


def kernel(**inputs):
    outp, _ = run(inputs)
    return outp


# revision 27
# speedup vs baseline: 1.0412x; 1.0412x over previous
"""Dense transformer block (B=4,S=2048,E=1024,H=16) on 8 trn2 cores.

Sharding: 2 cores per batch sequence; core parity p takes rows p, p+2, ...
(stride-2 interleave) as its query rows -- this balances causal-attention
work exactly across cores.  Each core's x input is row-permuted to
[q rows (local order), other-parity rows] so every SBUF offset in the
SPMD program is compile-time constant; causality is enforced with per-core
0/1 mask tensors (pure data).

All matmul operands are bf16 (full PE rate, half the DMA + SBUF of f32),
accumulation stays fp32 in PSUM.  K^T/Q^T/V/O all live in SBUF -- no DRAM
round trips for attention; FFN weights stream from HBM, overlapped.
"""

import numpy as np

B, S, E, H, DH = 4, 2048, 1024, 16, 64
EPS = 1e-5
QR = S // 2          # q rows per core
CH = 512             # q-chunk (matmul free dim)
NCH = QR // CH       # 2 chunks
NKB = S // 128       # 16 key blocks
ET = E // 128        # 8 E tiles
NPR = H // 2         # 8 head pairs
FE = 4 * E           # ffn hidden
NS4 = FE // 128      # 32 ffn hidden slices
SC = 1.0 / np.sqrt(DH)

# visit tables: per q-chunk, which key-blocks are fully visible / diagonal
FULL_KB = {0: [], 1: [0, 1, 2, 3, 8, 9, 10, 11]}
DIAG_KB = {0: [0, 1, 2, 3, 8, 9, 10, 11], 1: [4, 5, 6, 7, 12, 13, 14, 15]}

_PROG = None


def _build():
    import concourse.bacc as bacc
    import concourse.tile as tile
    from concourse import mybir
    from concourse.masks import make_identity

    F32 = mybir.dt.float32
    F32R = mybir.dt.float32r
    BF16 = mybir.dt.bfloat16
    AF = mybir.ActivationFunctionType
    SUB = mybir.AluOpType.subtract
    MULT = mybir.AluOpType.mult
    ADD = mybir.AluOpType.add

    nc = bacc.Bacc("TRN2", target_bir_lowering=False, debug=False, num_devices=8)

    xbf = nc.dram_tensor("xbf", [S, E], BF16, kind="ExternalInput").ap()
    xq = nc.dram_tensor("xq", [QR, E], F32, kind="ExternalInput").ap()
    masks2 = nc.dram_tensor("masks2", [2, 128, 2, 128], BF16, kind="ExternalInput").ap()
    wqh = nc.dram_tensor("wqh", [NPR, 128, ET, 128], BF16, kind="ExternalInput").ap()
    wkh = nc.dram_tensor("wkh", [NPR, 128, ET, 128], BF16, kind="ExternalInput").ap()
    wvh = nc.dram_tensor("wvh", [2, 128, ET, 8, DH], BF16, kind="ExternalInput").ap()
    woh = nc.dram_tensor("woh", [128, NPR, E], BF16, kind="ExternalInput").ap()
    w1h = nc.dram_tensor("w1h", [NS4, 128, ET, 128], BF16, kind="ExternalInput").ap()
    w2h = nc.dram_tensor("w2h", [2, NS4, 128, 512], BF16, kind="ExternalInput").ap()
    b2r = nc.dram_tensor("b2r", [1, E], BF16, kind="ExternalInput").ap()
    ln1g = nc.dram_tensor("ln1g", [128, ET], F32, kind="ExternalInput").ap()
    ln1b = nc.dram_tensor("ln1b", [128, ET], F32, kind="ExternalInput").ap()
    ln2g = nc.dram_tensor("ln2g", [128, ET], F32, kind="ExternalInput").ap()
    ln2b = nc.dram_tensor("ln2b", [128, ET], F32, kind="ExternalInput").ap()
    b1h = nc.dram_tensor("b1h", [128, NS4], F32, kind="ExternalInput").ap()
    out = nc.dram_tensor("out", [QR, E], F32, kind="ExternalOutput").ap()

    with tile.TileContext(nc, pool_alloc_mode="queue") as tc:
        consts = tc.alloc_tile_pool(name="consts", bufs=1)

        ident = consts.tile([128, 128], BF16, tag="ident")
        make_identity(nc, ident)
        onesb = consts.tile([1, 128], BF16, tag="onesb")
        nc.vector.memset(onesb, 1.0)
        epst = consts.tile([128, 1], F32, tag="eps")
        nc.vector.memset(epst, EPS)
        ln1g_sb = consts.tile([128, ET], F32, tag="lnp1")
        nc.sync.dma_start(ln1g_sb, ln1g)
        ln1b_sb = consts.tile([128, ET], F32, tag="lnp2")
        nc.sync.dma_start(ln1b_sb, ln1b)
        ln2g_sb = consts.tile([128, ET], F32, tag="lnp3")
        nc.sync.dma_start(ln2g_sb, ln2g)
        ln2b_sb = consts.tile([128, ET], F32, tag="lnp4")
        nc.sync.dma_start(ln2b_sb, ln2b)
        b1_sb = consts.tile([128, NS4], F32, tag="b1")
        nc.sync.dma_start(b1_sb, b1h)
        b2_sb = consts.tile([1, E], BF16, tag="b2")
        nc.sync.dma_start(b2_sb, b2r)
        wedges = []
        for w in range(2):
            mt = consts.tile([128, 2, 128], BF16, tag=f"wed{w}")
            nc.sync.dma_start(mt, masks2[w])
            wedges.append(mt)

        # persistent SBUF arrays (alloc order = reverse release order)
        ktp = tc.alloc_tile_pool(name="ktp", bufs=NPR)
        KT = [ktp.tile([128, S], BF16, tag="kt", name=f"KT{i}") for i in range(NPR)]
        qtp = tc.alloc_tile_pool(name="qtp", bufs=NPR)
        QT = [qtp.tile([128, QR], BF16, tag="qt", name=f"QT{i}") for i in range(NPR)]
        vap = tc.alloc_tile_pool(name="vap", bufs=NKB)
        VA = [vap.tile([128, H, DH + 1], BF16, tag="va", name=f"VA{i}")
              for i in range(NKB)]
        y2p = tc.alloc_tile_pool(name="y2p", bufs=ET, side="right")
        Y2T = [y2p.tile([128, QR], BF16, tag="y2t", name=f"Y2T{i}")
               for i in range(ET)]
        oap = tc.alloc_tile_pool(name="oap", bufs=NPR)
        OACC = [oap.tile([128, QR], BF16, tag="oacc", name=f"OACC{i}")
                for i in range(NPR)]
        dramp = tc.alloc_tile_pool(name="dramp", bufs=1, space="DRAM")
        X2D = dramp.tile([QR, E], F32, name="X2D")

        for kb in range(NKB):
            nc.vector.memset(VA[kb][:, :, DH], 1.0)  # softmax-denominator column

        # ---------------- P1: LN1 + transpose + QKV projections, streamed ----
        with (
            tc.tile_pool(name="wqkv", bufs=1) as wqkv,
            tc.tile_pool(name="xp", bufs=8) as xp,
            tc.tile_pool(name="y1p", bufs=2) as y1p,
            tc.tile_pool(name="psT", bufs=2, space="PSUM") as psT,
            tc.tile_pool(name="psP", bufs=3, space="PSUM") as psP,
        ):
            wvs = []
            for half in range(2):
                wt = wqkv.tile([128, ET, 8, DH], BF16, tag="wv", name=f"wv{half}",
                               bufs=2)
                nc.sync.dma_start(wt, wvh[half])
                wvs.append(wt)

            for tq in range(4):
                xts = []
                mvall = xp.tile([128, 4, 2], F32, tag="bnmv", bufs=2)
                for j in range(4):
                    xt = xp.tile([128, E], BF16, tag="xt", name=f"xt{j}", bufs=6)
                    nc.sync.dma_start(
                        xt, xbf[(tq * 4 + j) * 128:(tq * 4 + j + 1) * 128, :])
                    st = xp.tile([128, 2, 6], F32, tag="bnst", bufs=4)
                    xr = xt.rearrange("p (a b) -> p a b", a=2)
                    for sg in range(2):
                        nc.vector.bn_stats(st[:, sg, :], xr[:, sg, :])
                    nc.vector.bn_aggr(mvall[:, j, :], st)
                    xts.append(xt)
                sq = xp.tile([128, 4], F32, tag="sq", bufs=2)
                nc.scalar.activation(sq, mvall[:, :, 1], AF.Sqrt, bias=epst)
                rstd = xp.tile([128, 4], F32, tag="rstd", bufs=2)
                nc.vector.reciprocal_approx_fast(rstd, sq)
                for j in range(4):
                    nc.vector.tensor_scalar(
                        xts[j], xts[j], mvall[:, j, 0:1], rstd[:, j:j + 1],
                        SUB, MULT)
                if tq == 0:
                    # warm the HAM clock gate between the LN head and the
                    # first projection matmuls (depends on xts[0] so it
                    # doesn't fire too early and re-throttle before use)
                    warm = psT.tile([128, 1024], BF16, tag="pst")
                    for _ in range(80):
                        nc.tensor.transpose(warm[:, 0:128],
                                            xts[0][:, 0:128], ident)
                y1t = y1p.tile([128, ET, 512], BF16, tag="y1t")
                for e in range(ET):
                    pst = psT.tile([128, 1024], BF16, tag="pst")
                    for j in range(4):
                        nc.tensor.transpose(
                            pst[:, j * 128:(j + 1) * 128],
                            xts[j][:, e * 128:(e + 1) * 128], ident)
                    nc.vector.tensor_scalar(
                        y1t[:, e, :], pst[:, 0:512],
                        ln1g_sb[:, e:e + 1], ln1b_sb[:, e:e + 1], MULT, ADD)
                # K projection for this 512-col block of all head pairs
                for pr in range(NPR):
                    wk_t = wqkv.tile([128, ET, 128], BF16, tag="wk", bufs=4)
                    nc.sync.dma_start(wk_t, wkh[pr])
                    ps = psP.tile([128, 512], F32, tag="pp")
                    for e in range(ET):
                        nc.tensor.matmul(ps, wk_t[:, e, :], y1t[:, e, :],
                                         start=(e == 0), stop=(e == ET - 1))
                    nc.scalar.copy(KT[pr][:, tq * 512:(tq + 1) * 512], ps)
                # Q projection (first two 512-blocks are the q rows)
                if tq < NCH:
                    for pr in range(NPR):
                        wq_t = wqkv.tile([128, ET, 128], BF16, tag="wq", bufs=4)
                        nc.sync.dma_start(wq_t, wqh[pr])
                        ps = psP.tile([128, 512], F32, tag="pp")
                        for e in range(ET):
                            nc.tensor.matmul(ps, wq_t[:, e, :], y1t[:, e, :],
                                             start=(e == 0), stop=(e == ET - 1))
                        nc.vector.tensor_copy(QT[pr][:, tq * 512:(tq + 1) * 512], ps)
                # V projection (natural layout) for the 4 key blocks here
                for kbl in range(4):
                    kb = 4 * tq + kbl
                    for half in range(2):
                        ps = psP.tile([128, 512], F32, tag="pp")
                        for e in range(ET):
                            nc.tensor.matmul(
                                ps, y1t[:, e, kbl * 128:(kbl + 1) * 128],
                                wvs[half][:, e, :, :],
                                start=(e == 0), stop=(e == ET - 1))
                        nc.scalar.copy(
                            VA[kb][:, 8 * half:8 * half + 8, 0:DH],
                            ps.rearrange("p (h d) -> p h d", h=8))

        # ---------------- P2: attention;  P3: out proj;  P4: LN2 ----------
        with tc.tile_pool(name="wop", bufs=1) as wop:
            wo_sb = wop.tile([128, NPR, E], BF16, tag="wo")
            nc.sync.dma_start(wo_sb, woh)

            with (
                tc.tile_pool(name="ptp", bufs=4) as ptp,
                tc.tile_pool(name="nrm", bufs=2) as nrm,
                tc.tile_pool(name="psS", bufs=2, space="PSUM") as psS,
                tc.tile_pool(name="psO", bufs=4, space="PSUM") as psO,
            ):
                def do_norm(ops, ch, pr):
                    # normalize: o / rowsum -> OACC.  Runs inline at each
                    # stream's pair boundary; the other stream's visits keep
                    # the PE busy during this vector chain.
                    rs = nrm.tile([1, 1024], F32, tag="rs")
                    for hh in range(2):
                        nc.vector.tensor_copy(
                            rs[0:1, hh * 512:(hh + 1) * 512],
                            ops[hh][DH:DH + 1, :])
                    rcf = nrm.tile([1, 1024], F32, tag="rcf")
                    nc.vector.reciprocal_approx_fast(rcf, rs)
                    bcs = nrm.tile([64, 1024], F32, tag="bcs")
                    nc.gpsimd.partition_broadcast(bcs, rcf[0:1, :])
                    for hh in range(2):
                        nc.vector.tensor_mul(
                            OACC[pr][hh * 64:(hh + 1) * 64,
                                     ch * 512:(ch + 1) * 512],
                            ops[hh][0:DH, :], bcs[:, hh * 512:(hh + 1) * 512])

                def do_post(item):
                    ops, ch, pr, kb, q0, N, pss, pt, first, last = item
                    if N == 512:
                        nc.scalar.activation(pt, pss, AF.Exp, scale=SC)
                    else:
                        pss3 = pss.rearrange("p (a b) -> p a b", a=2)[:, :, 0:N]
                        pt3 = pt.rearrange("p (a b) -> p a b", a=2)[:, :, 0:N]
                        nc.scalar.activation(pt3, pss3, AF.Exp, scale=SC)
                    if kb in DIAG_KB[ch]:
                        ptm = pt.rearrange("p (a b) -> p a b", a=2)[:, :, 0:128]
                        nc.vector.tensor_mul(ptm, ptm,
                                             wedges[0 if kb < 8 else 1])
                    for hh in range(2):
                        nc.tensor.matmul(
                            ops[hh][0:DH + 1, q0:512],
                            VA[kb][:, 2 * pr + hh, :],
                            pt[:, hh * 512:hh * 512 + N],
                            start=first, stop=last, skip_group_check=True)

                # single stream per (ch, pair); normalize of the previous
                # pair is deferred into visit 4 of the next pair's stream so
                # the PE never idles long enough for HAM to re-throttle.
                pending_norm = None
                pending = None
                for ch in range(NCH):
                    visits = FULL_KB[ch] + DIAG_KB[ch]
                    nv = len(visits)
                    for pr in range(NPR):
                        ops = [psO.tile([128, 512], F32, tag="ot",
                                        name=f"ot{h}") for h in range(2)]
                        for vi, kb in enumerate(visits):
                            diag = kb in DIAG_KB[ch]
                            q0 = 128 * (kb % 4) if diag else 0
                            N = 512 - q0
                            pss = psS.tile([128, 1024], F32, tag="sc")
                            for hh in range(2):
                                nc.tensor.matmul(
                                    pss[:, hh * 512:hh * 512 + N],
                                    KT[pr][hh * 64:(hh + 1) * 64,
                                           kb * 128:(kb + 1) * 128],
                                    QT[pr][hh * 64:(hh + 1) * 64,
                                           ch * 512 + q0:(ch + 1) * 512],
                                    start=True, stop=True)
                            pt = ptp.tile([128, 1024], BF16, tag="pt", bufs=4)
                            if pending is not None:
                                do_post(pending)
                            pending = (ops, ch, pr, kb, q0, N, pss, pt,
                                       vi == 0, vi == nv - 1)
                            if vi == 4 and pending_norm is not None:
                                do_norm(*pending_norm)
                                pending_norm = None
                        pending_norm = (ops, ch, pr)
                do_post(pending)
                do_norm(*pending_norm)

            # ---- P3 (out projection + residual) with inline P4 (LN2) ----
            with (
                tc.tile_pool(name="xqp", bufs=2) as xqp,
                tc.tile_pool(name="x2tp", bufs=6) as x2tp,
                tc.tile_pool(name="n2p", bufs=4) as n2p,
                tc.tile_pool(name="st2", bufs=4) as st2,
                tc.tile_pool(name="psP3", bufs=3, space="PSUM") as psP3,
                tc.tile_pool(name="psT2", bufs=2, space="PSUM") as psT2,
            ):
                x2group = []

                def ln2_half(tq):
                    n2s = []
                    mvall = st2.tile([128, 4, 2], F32, tag="bnmv2", bufs=2)
                    for j in range(4):
                        x2t = x2group[j]
                        st = st2.tile([128, 2, 6], F32, tag="bnst2")
                        x2r = x2t.rearrange("p (a b) -> p a b", a=2)
                        for sg in range(2):
                            nc.vector.bn_stats(st[:, sg, :], x2r[:, sg, :])
                        nc.vector.bn_aggr(mvall[:, j, :], st)
                    sq = st2.tile([128, 4], F32, tag="sq2", bufs=2)
                    nc.scalar.activation(sq, mvall[:, :, 1], AF.Sqrt, bias=epst)
                    rstd = st2.tile([128, 4], F32, tag="rstd2", bufs=2)
                    nc.vector.reciprocal_approx_fast(rstd, sq)
                    for j in range(4):
                        n2 = n2p.tile([128, E], BF16, tag="n2", name=f"n2{j}")
                        nc.vector.tensor_scalar(
                            n2, x2group[j], mvall[:, j, 0:1], rstd[:, j:j + 1],
                            SUB, MULT)
                        n2s.append(n2)
                    for e in range(ET):
                        pst = psT2.tile([128, 1024], BF16, tag="pst2")
                        for j in range(4):
                            nc.tensor.transpose(
                                pst[:, j * 128:(j + 1) * 128],
                                n2s[j][:, e * 128:(e + 1) * 128], ident)
                        nc.vector.tensor_scalar(
                            Y2T[e][:, tq * 512:(tq + 1) * 512], pst[:, 0:512],
                            ln2g_sb[:, e:e + 1], ln2b_sb[:, e:e + 1], MULT, ADD)

                for qt in range(ET):
                    xq_t = xqp.tile([128, E], F32, tag="xq")
                    nc.sync.dma_start(xq_t, xq[qt * 128:(qt + 1) * 128, :])
                    x2t = x2tp.tile([128, E], F32, tag="x2t")
                    for eh in range(2):
                        ps = psP3.tile([128, 512], F32, tag="po")
                        for pr in range(NPR):
                            nc.tensor.matmul(
                                ps, OACC[pr][:, qt * 128:(qt + 1) * 128],
                                wo_sb[:, pr, eh * 512:(eh + 1) * 512],
                                start=(pr == 0), stop=(pr == NPR - 1))
                        nc.vector.tensor_add(
                            x2t[:, eh * 512:(eh + 1) * 512], ps,
                            xq_t[:, eh * 512:(eh + 1) * 512])
                    nc.sync.dma_start(X2D[qt * 128:(qt + 1) * 128, :], x2t)
                    x2group.append(x2t)
                    if qt == 3:
                        ln2_half(0)
                        x2group = []
                ln2_half(1)

        oap.release()
        vap.release()
        qtp.release()
        ktp.release()

        # ---------------- P5: FFN + residual -> out ----------------
        with (
            tc.tile_pool(name="h1p", bufs=NS4) as h1p,
            tc.tile_pool(name="w1p", bufs=8) as w1p,
            tc.tile_pool(name="w2p", bufs=32) as w2p,
            tc.tile_pool(name="otp", bufs=4) as otp,
            tc.tile_pool(name="x2rp", bufs=8) as x2rp,
            tc.tile_pool(name="psF1", bufs=2, space="PSUM") as psF1,
            tc.tile_pool(name="psF2", bufs=4, space="PSUM") as psF2,
        ):
            for qh in range(NCH):
                h1t = []
                for s4 in range(NS4):
                    w1s = w1p.tile([128, ET, 128], BF16, tag="w1")
                    nc.sync.dma_start(w1s, w1h[s4])
                    ps = psF1.tile([128, 512], F32, tag="f1")
                    for e in range(ET):
                        nc.tensor.matmul(
                            ps, w1s[:, e, :], Y2T[e][:, qh * 512:(qh + 1) * 512],
                            start=(e == 0), stop=(e == ET - 1))
                    ht = h1p.tile([128, 512], BF16, tag="h1")
                    nc.scalar.activation(ht, ps, AF.Relu,
                                         bias=b1_sb[:, s4:s4 + 1])
                    h1t.append(ht)
                for eh in range(2):
                    x2rs = []
                    for ql in range(4):
                        qt = qh * 4 + ql
                        x2r = x2rp.tile([128, 512], F32, tag="x2r")
                        nc.sync.dma_start(
                            x2r, X2D[qt * 128:(qt + 1) * 128,
                                     eh * 512:(eh + 1) * 512])
                        x2rs.append(x2r)
                    pss_t = [psF2.tile([128, 512], F32, tag="f2", name=f"f2_{q}")
                             for q in range(4)]
                    for s4 in range(NS4):
                        w2s = w2p.tile([128, 512], BF16, tag="w2")
                        nc.sync.dma_start(w2s, w2h[eh, s4])
                        for ql in range(4):
                            nc.tensor.matmul(
                                pss_t[ql], h1t[s4][:, ql * 128:(ql + 1) * 128],
                                w2s, start=(s4 == 0), stop=False)
                    for ql in range(4):
                        qt = qh * 4 + ql
                        nc.tensor.matmul(
                            pss_t[ql], onesb, b2_sb[0:1, eh * 512:(eh + 1) * 512],
                            start=False, stop=True)
                        ot = otp.tile([128, 512], F32, tag="ot")
                        nc.vector.tensor_add(ot, pss_t[ql], x2rs[ql])
                        nc.sync.dma_start(
                            out[qt * 128:(qt + 1) * 128,
                                eh * 512:(eh + 1) * 512], ot)

        y2p.release()
        dramp.release()
        consts.release()

    nc.compile()
    return nc


def _prep_inputs(inputs):
    import ml_dtypes
    bf16 = ml_dtypes.bfloat16

    x = np.ascontiguousarray(inputs["x"], dtype=np.float32)
    wq = np.ascontiguousarray(inputs["Wq"], np.float32)   # [H, E, DH]
    wk = np.ascontiguousarray(inputs["Wk"], np.float32)
    wv = np.ascontiguousarray(inputs["Wv"], np.float32)
    wo = np.ascontiguousarray(inputs["Wo"], np.float32)   # [E, E]
    bo = np.ascontiguousarray(inputs["bo"], np.float32)
    w1 = np.ascontiguousarray(inputs["W1"], np.float32)   # [E, 4E]
    b1 = np.ascontiguousarray(inputs["b1"], np.float32)
    w2 = np.ascontiguousarray(inputs["W2"], np.float32)   # [4E, E]
    b2 = np.ascontiguousarray(inputs["b2"], np.float32)

    def qkv_pack(w, group):  # [H, E, DH] -> [ngrp, 128, ET, group*DH]
        n = H // group
        a = w.reshape(n, group, ET, 128, DH)          # [n, g, eo, ei, d]
        a = a.transpose(0, 3, 2, 1, 4)                # [n, ei, eo, g, d]
        return np.ascontiguousarray(
            a.reshape(n, 128, ET, group * DH)).astype(bf16)

    shared = {
        "wqh": qkv_pack(wq, 2),
        "wkh": qkv_pack(wk, 2),
        "wvh": qkv_pack(wv, 8),
        "woh": np.ascontiguousarray(
            wo.reshape(NPR, 128, E).transpose(1, 0, 2)).astype(bf16),
        "w1h": np.ascontiguousarray(
            w1.reshape(ET, 128, NS4, 128).transpose(2, 1, 0, 3)).astype(bf16),
        "w2h": np.ascontiguousarray(
            w2.reshape(NS4, 128, 2, 512).transpose(2, 0, 1, 3)).astype(bf16),
        "b2r": b2.reshape(1, E).astype(bf16),
        "b1h": np.ascontiguousarray(b1.reshape(NS4, 128).T),
        "ln1g": np.ascontiguousarray(inputs["ln1_g"].reshape(ET, 128).T),
        "ln1b": np.ascontiguousarray(inputs["ln1_b"].reshape(ET, 128).T),
        "ln2g": np.ascontiguousarray(inputs["ln2_g"].reshape(ET, 128).T),
        "ln2b": np.ascontiguousarray(inputs["ln2_b"].reshape(ET, 128).T),
    }
    kk = np.arange(128)[:, None]
    qq = np.arange(128)[None, :]
    in_maps = []
    for c in range(8):
        b, p = c // 2, c % 2
        perm = np.concatenate([np.arange(p, S, 2), np.arange(1 - p, S, 2)])
        m = np.zeros((2, 128, 128), np.float32)
        m[0] = (qq >= kk).astype(np.float32)          # own-parity blocks
        if p == 0:
            m[1] = (qq > kk).astype(np.float32)       # other-parity, even core
        else:
            m[1] = (qq >= kk).astype(np.float32)      # other-parity, odd core
        im = dict(shared)
        im["xbf"] = np.ascontiguousarray(x[b][perm]).astype(bf16)
        im["xq"] = np.ascontiguousarray(x[b][perm[:QR]] + bo[None, :])
        im["masks2"] = np.ascontiguousarray(
            np.broadcast_to(m[:, :, None, :], (2, 128, 2, 128))).astype(bf16)
        in_maps.append(im)
    return in_maps


def _get_prog():
    global _PROG
    if _PROG is None:
        _PROG = _build()
    return _PROG


def run(inputs, trace=False):
    from concourse.bass_utils import run_bass_kernel_spmd

    nc = _get_prog()
    in_maps = _prep_inputs(inputs)
    kw = {}
    if trace:
        import sys, types
        try:
            from antenv.axon_hooks import get_axon_ntff_profile_hook  # noqa
        except ImportError:
            from trn_agent_boot.trn_boot import _ntff_profile_via_ctypes
            hook = _ntff_profile_via_ctypes("/opt/axon/libaxon_pjrt.so")
            mod = types.ModuleType("antenv.axon_hooks")
            mod.get_axon_ntff_profile_hook = lambda: hook
            sys.modules["antenv.axon_hooks"] = mod
        kw["trace"] = True
    res = run_bass_kernel_spmd(nc, in_maps, core_ids=list(range(8)), **kw)
    outp = np.empty((B, S, E), np.float32)
    for c in range(8):
        b, p = c // 2, c % 2
        outp[b, p::2, :] = res.results[c]["out"]
    return outp, res


def kernel(**inputs):
    outp, _ = run(inputs)
    return outp
